# revision 10
# baseline (speedup 1.0000x reference)
"""CoAttention kernel for 8 TRN2 NeuronCores (Bass/Tile, SPMD).

Problem: B=4 batches x 2 attention directions = 8 independent co-attention
computations -> one per core.  Per core (batch b, direction d):
    Q = wq @ qf + bq        [256, 2304]     (qf = query-side features)
    K = wr @ rf + br        [256, 2304]     (rf = reference-side features)
    S^T = K^T Q             [2304, 2304]    (computed in m-strips of 128)
    attnT = exp(S^T - 40)   (bf16, unnormalized; softmax denom applied at end)
    sums[q] = sum_m attnT[m, q]
    outT = (attnT^T @ rf^T) * (1/sums)      [2304, 2048]  (host transposes)
Host assembles: left_att = concat(left, out[b,dir=0]), right_att likewise.

Precision: score path in float32r (RNE-11-mantissa, full PE rate at free>=256,
host pre-rounds inputs), attn@V in bf16.  No row-max subtraction: scores are
|S| <~ 80, exp(S-40) stays in fp32/bf16 range; normalization is exact math.

Schedule (v2): the PE instruction stream is kept dense end-to-end:
 - weights host-pre-permuted to [128, CB*D] so DMA rows are contiguous per
   partition; first weight/x blocks loaded as small DMAs so matmuls start ~9us
   instead of ~26us; loads split across sync (xq), scalar (xr, vt), gpsimd
   (w, stores) queues.
 - S^T strips for qt=0 are interleaved into the projection phase (they have no
   DMA dependency and fill x-stream stall gaps); strips for qt+1 are
   interleaved into av(qt) chains.
 - attn@V computed transposed: lhsT = attnT strip (weight), rhs = V^T strip
   (moving) -> outT [q, c] in 6 q-blocks x 4 c-chunks of 512, every matmul at
   full 512 moving columns.  Normalization is then a per-PARTITION scalar
   multiply: sums come from 6 single-column matmuls against the DVE-maintained
   strip-sum accumulator, and 1/sums is a [128,6] DVE reciprocal (~50ns)
   instead of a [1,768] single-lane one (~5us).  No broadcast tile, no DRAM
   round-trip.

Walrus in this toolchain allows ONE sync-wait per instruction; SafeTileContext
splits multi-wait instructions into standalone wait ops, and splits the
end-of-kernel drain the same way.
"""
import numpy as np
import ml_dtypes

import concourse.bass as bass
import concourse.mybir as mybir
import concourse.tile as tile
from concourse.vector_clock import ScopedClock
from concourse.bass_utils import run_bass_kernel_spmd

B = 4
C = 2048
HW = 48 * 48          # 2304
D = 256
NCORES = 8

CB = C // 128         # 16 c-blocks
DB = D // 128         # 2 d-blocks
MS = HW // 128        # 18 m-strips
# phase-1 n chunks: 512-wide (max f32r moving free dim) + 256 tail;
# each chunk's psum fits one 2KB PSUM bank
P1CHUNKS = [(0, 512), (512, 512), (1024, 512), (1536, 512), (2048, 256)]
NQT = 3               # phase-2 q thirds
QT = HW // NQT        # 768
QB = QT // 128        # 6 q-blocks per third
NCC = C // 512        # 4 c-chunks per av output row-block
# sub-chunks within a q-third for the score matmuls (a matmul output must not
# cross a 2KB PSUM bank boundary; both >=256 keeps f32r at full rate)
SUBS = [(0, 512), (512, 256)]

# S^T strips of qt=0 interleaved into projection: (chunk, pair) -> strips
# emitted before that x-pair's matmuls (strip m needs K columns from chunk
# m*128//512, and Q columns 0:768 from chunks 0-1)
STRIPS_AT = {
    (2, 0): (0,), (2, 1): (1,), (2, 2): (2,), (2, 3): (3,),
    (2, 4): (4,), (2, 5): (5,), (2, 6): (6,), (2, 7): (7,),
    (3, 0): (8,), (3, 2): (9,), (3, 4): (10,), (3, 6): (11,),
    (4, 0): (12,), (4, 2): (13,), (4, 4): (14,), (4, 6): (15,),
}
# vt stripe DMAs emitted after these (chunk, pair)'s matmuls (on the scalar
# queue, behind that chunk's xr loads, so the DMA engines' descriptor FIFOs
# naturally prioritize the x stream)
VT_AT = {(1, 2): 0, (1, 5): 1, (2, 2): 2, (2, 5): 3, (3, 2): 4, (3, 5): 5}

F32 = mybir.dt.float32
F32R = mybir.dt.float32r
BF16 = mybir.dt.bfloat16

# module-level knobs / results (used by test.py)
TRACE = False
LAST_RESULT = None


class SafeTileContext(tile.TileContext):
    """This walrus build allows at most ONE sync wait per instruction.
    Hoist extra waits onto standalone EventSemaphore (wait-only) ops placed
    immediately before, on the same engine queue; same for the final drain."""
    MAX_WAITS = 1

    def _lower_ordered_insts(self, ordered):
        for bname, insts in ordered.items():
            new_list = []
            for inst in insts:
                si = inst.sync_info
                if si is not None and len(si.on_wait) > self.MAX_WAITS:
                    waits = list(si.on_wait)
                    movable = [w for w in waits if w.wait_reg is None]
                    fixed = [w for w in waits if w.wait_reg is not None]
                    keep = fixed + movable[-1:] if movable else fixed
                    hoist = movable[:-1] if movable else []
                    for w in hoist:
                        wi = mybir.InstEventSemaphore(
                            name=self.nc.get_next_instruction_name(),
                            ins=[], outs=[])
                        wi.engine = inst.engine
                        wi.sync_info = mybir.SyncInfo(on_wait=[w], on_update=[])
                        new_list.append(wi)
                    inst.sync_info = mybir.SyncInfo(
                        on_wait=keep, on_update=list(si.on_update))
                new_list.append(inst)
            insts[:] = new_list
        super()._lower_ordered_insts(ordered)

    def _drain_and_barrier(self, tick_clock, wait_clock):
        drain_inst = self.nc.sync.drain()
        wait_clock.add_sem_waits(
            drain_inst.ins, ScopedClock({None: tick_clock.global_clock}))
        si = drain_inst.ins.sync_info
        waits = list(si.on_wait) if si is not None else []
        ups = list(si.on_update) if si is not None else []
        if len(waits) > self.MAX_WAITS:
            drain_inst.ins.sync_info = mybir.SyncInfo(
                on_wait=waits[: self.MAX_WAITS], on_update=ups)
            rest = waits[self.MAX_WAITS:]
            for i in range(0, len(rest), self.MAX_WAITS):
                extra = self.nc.sync.drain()
                extra.ins.sync_info = mybir.SyncInfo(
                    on_wait=rest[i : i + self.MAX_WAITS], on_update=[])
        self.nc.all_engine_barrier()
        assert self.sems is not None
        popped = self.nc._tile_sem_poison_stack.pop()
        assert popped is self._sem_poison
        self.nc.clear_and_free_semaphores(list(self.sems.allocated().values()))
        self.nc.all_engine_barrier()


def build_kernel():
    nc = bass.Bass("TRN2", target_bir_lowering=False, debug=False)

    qf = nc.dram_tensor("qf", [C, HW], F32R, kind="ExternalInput")
    rf = nc.dram_tensor("rf", [C, HW], F32R, kind="ExternalInput")
    vtb = nc.dram_tensor("vtb", [HW, C], BF16, kind="ExternalInput")
    # weights host-permuted to [128, CB*D]: row p holds wq.T[k*128+p, :] for
    # k = 0..CB-1 -> per-partition contiguous DMA rows
    wqp = nc.dram_tensor("wqp", [128, CB * D], F32R, kind="ExternalInput")
    wrp = nc.dram_tensor("wrp", [128, CB * D], F32R, kind="ExternalInput")
    bq = nc.dram_tensor("bq", [128, DB], F32, kind="ExternalInput")
    br = nc.dram_tensor("br", [128, DB], F32, kind="ExternalInput")
    outT = nc.dram_tensor("outT", [HW, C], F32, kind="ExternalOutput")

    with SafeTileContext(nc) as tc:
        with tc.tile_pool(name="persist", bufs=1) as persist, \
             tc.tile_pool(name="attnpA", bufs=1) as attnpA, \
             tc.tile_pool(name="spsp", bufs=2, space="PSUM") as spsp:
            # ---- persistent tiles ----
            q_sb = persist.tile([128, DB, HW], F32R)    # Q  [d, n]
            k_sb = persist.tile([128, DB, HW], F32R)    # K  [d, n]
            vt = persist.tile([128, MS, C], BF16)       # V^T [m, c]
            bq_t = persist.tile([128, DB], F32)
            br_t = persist.tile([128, DB], F32)
            nc.sync.dma_start(out=bq_t, in_=bq.ap())
            nc.sync.dma_start(out=br_t, in_=br.ap())
            nbias = persist.tile([128, 1], F32)
            nc.vector.memset(nbias, -40.0)
            ones_col = persist.tile([128, 1], BF16)     # sums reduce rhs
            nc.vector.memset(ones_col, 1.0)
            partial = persist.tile([128, QT], F32)      # DVE strip-accumulator
            partial_r = persist.tile([128, QT], BF16)   # PE copy (free=1
            # matmuls are bf16: f32r is illegal there; one 2^-9 rounding of
            # the softmax denominator, ~0.2% common-mode, is well in budget)

            attn_cur = {}
            invs_cur = {}
            pools = {}   # filled once the phase-2 pools open

            def emit_strip(qt, m):
                """Score matmuls + exp + DVE partial-sum for one m-strip."""
                if m == 0:
                    pool = attnpA if qt % 2 == 0 else pools["attnpB"]
                    attn_cur[qt] = pool.tile(
                        [128, MS, QT], BF16,
                        tag="attnA" if qt % 2 == 0 else "attnB",
                        name=f"attnT_{qt}")
                attn_t = attn_cur[qt]
                sps = spsp.tile([128, QT], F32, tag="sps", name=f"sps_{qt}_{m}")
                for off, sz in SUBS:
                    for dd in range(DB):
                        nc.tensor.matmul(
                            sps[:, off:off + sz],
                            k_sb[:, dd, m * 128:(m + 1) * 128],
                            q_sb[:, dd, qt * QT + off:qt * QT + off + sz],
                            start=(dd == 0), stop=(dd == DB - 1))
                nc.scalar.activation(
                    attn_t[:, m, :], sps,
                    mybir.ActivationFunctionType.Exp,
                    bias=nbias, scale=1.0)
                if m == 0:
                    nc.vector.tensor_copy(partial, attn_t[:, m, :])
                else:
                    nc.vector.tensor_add(partial, attn_t[:, m, :], partial)

            def emit_sums(qt):
                """partial [128,QT] -> per-q-block sums [128q, 1] (6 tiny
                matmuls) -> 1/sums [128, QB] via one cheap DVE reciprocal."""
                opsp = pools["opsp"]
                small = pools["small"]
                nc.vector.tensor_copy(partial_r, partial)
                aux = opsp.tile([128, 8], F32, tag="aux", name=f"aux_{qt}")
                for qb in range(QB):
                    nc.tensor.matmul(
                        aux[:, qb:qb + 1],
                        partial_r[:, qb * 128:(qb + 1) * 128],
                        ones_col, start=True, stop=True)
                invs_t = small.tile([128, 8], F32, tag="invs",
                                    name=f"invs_{qt}")
                nc.vector.reciprocal(invs_t[:, 0:QB], aux[:, 0:QB])
                invs_cur[qt] = invs_t

            # ================= phase 1: projections + S^T(qt0) ============
            with tc.tile_pool(name="wpool", bufs=1) as wpool, \
                 tc.tile_pool(name="xstream", bufs=2) as xstream, \
                 tc.tile_pool(name="p1ps", bufs=1, space="PSUM") as p1ps:
                wq_sb = wpool.tile([128, CB, D], F32R)
                wr_sb = wpool.tile([128, CB, D], F32R)
                # weights on gpsimd queue, first blocks small so matmuls can
                # start immediately
                for c0, c1 in ((0, 2), (2, 4), (4, 10), (10, 16)):
                    nc.gpsimd.dma_start(out=wq_sb[:, c0:c1, :],
                                        in_=wqp.ap()[:, c0 * D:c1 * D])
                    nc.gpsimd.dma_start(out=wr_sb[:, c0:c1, :],
                                        in_=wrp.ap()[:, c0 * D:c1 * D])

                qfr = qf.ap().rearrange("(k p) n -> p k n", p=128)
                rfr = rf.ap().rearrange("(k p) n -> p k n", p=128)
                vtr = vtb.ap().rearrange("(s p) c -> p s c", p=128)

                for ch, (coff, csz) in enumerate(P1CHUNKS):
                    qps = [p1ps.tile([128, 512], F32, tag=f"qps{d}",
                                     name=f"qps{d}_{ch}")
                           for d in range(DB)]
                    kps = [p1ps.tile([128, 512], F32, tag=f"kps{d}",
                                     name=f"kps{d}_{ch}")
                           for d in range(DB)]
                    for pair in range(CB // 2):
                        xq = xstream.tile([128, 2, 512], F32R, tag="xq",
                                          name=f"xq_{ch}_{pair}")
                        xr = xstream.tile([128, 2, 512], F32R, tag="xr",
                                          name=f"xr_{ch}_{pair}")
                        nc.sync.dma_start(
                            out=xq[:, :, :csz],
                            in_=qfr[:, pair * 2:(pair + 1) * 2,
                                    coff:coff + csz])
                        nc.scalar.dma_start(
                            out=xr[:, :, :csz],
                            in_=rfr[:, pair * 2:(pair + 1) * 2,
                                    coff:coff + csz])
                        for m in STRIPS_AT.get((ch, pair), ()):
                            emit_strip(0, m)
                        for i in range(2):
                            c = pair * 2 + i
                            for d in range(DB):
                                nc.tensor.matmul(
                                    qps[d][:, :csz],
                                    wq_sb[:, c, d * 128:(d + 1) * 128],
                                    xq[:, i, :csz],
                                    start=(c == 0), stop=(c == CB - 1))
                                nc.tensor.matmul(
                                    kps[d][:, :csz],
                                    wr_sb[:, c, d * 128:(d + 1) * 128],
                                    xr[:, i, :csz],
                                    start=(c == 0), stop=(c == CB - 1))
                        h = VT_AT.get((ch, pair))
                        if h is not None:
                            nc.scalar.dma_start(
                                out=vt[:, 3 * h:3 * (h + 1), :],
                                in_=vtr[:, 3 * h:3 * (h + 1), :])
                    # k-side bias first: the last S^T strips wait on it
                    for d in range(DB):
                        nc.vector.tensor_scalar_add(
                            k_sb[:, d, coff:coff + csz],
                            kps[d][:, :csz], br_t[:, d:d + 1])
                    for d in range(DB):
                        nc.vector.tensor_scalar_add(
                            q_sb[:, d, coff:coff + csz],
                            qps[d][:, :csz], bq_t[:, d:d + 1])
                # last strips need the final chunk's bias-adds
                emit_strip(0, 16)
                emit_strip(0, 17)

            # ================= phase 2: av(qt) + S^T(qt+1) ================
            with tc.tile_pool(name="attnpB", bufs=1) as attnpB, \
                 tc.tile_pool(name="small", bufs=2) as small, \
                 tc.tile_pool(name="ostage", bufs=3) as ostage, \
                 tc.tile_pool(name="opsp", bufs=2, space="PSUM") as opsp:
                pools["attnpB"] = attnpB
                pools["small"] = small
                pools["opsp"] = opsp

                # sums(0) (brief PE wait on the last strip's exp+add), then
                # 7 strips of qt1 emitted before av(0)
                emit_sums(0)
                for j in range(7):
                    emit_strip(1, j)

                def av_phase(qt, hooks):
                    attn_t = attn_cur.pop(qt)
                    invs_t = invs_cur.pop(qt)
                    last = (qt == NQT - 1)
                    ci = 0
                    for qb in range(QB):
                        oT = ostage.tile([128, C], F32, tag="osb",
                                         name=f"osb_{qt}_{qb}")
                        for cc in range(NCC):
                            for fn in hooks.get(ci, ()):
                                fn()
                            ci += 1
                            ops = opsp.tile([128, 512], F32, tag="ops")
                            for m in range(MS):
                                nc.tensor.matmul(
                                    ops,
                                    attn_t[:, m, qb * 128:(qb + 1) * 128],
                                    vt[:, m, cc * 512:(cc + 1) * 512],
                                    start=(m == 0), stop=(m == MS - 1))
                            nc.vector.tensor_scalar_mul(
                                oT[:, cc * 512:(cc + 1) * 512], ops,
                                invs_t[:, qb:qb + 1])
                            if last and qb >= 4:
                                nc.gpsimd.dma_start(
                                    out=outT.ap()[qt * QT + qb * 128:
                                                  qt * QT + (qb + 1) * 128,
                                                  cc * 512:(cc + 1) * 512],
                                    in_=oT[:, cc * 512:(cc + 1) * 512])
                        if not (last and qb >= 4):
                            nc.gpsimd.dma_start(
                                out=outT.ap()[qt * QT + qb * 128:
                                              qt * QT + (qb + 1) * 128, :],
                                in_=oT)

                # av(0): remaining strips of qt1 at chains 1..11, sums(1) at
                # 14 (after qt1 exps/adds land, invs ready long before av(1))
                hooks0 = {}
                for j in range(7, MS):
                    hooks0.setdefault(1 + (j - 7), []).append(
                        lambda m=j: emit_strip(1, m))
                hooks0.setdefault(14, []).append(lambda: emit_sums(1))
                av_phase(0, hooks0)

                hooks1 = {}
                for j in range(MS):
                    hooks1.setdefault(1 + j, []).append(
                        lambda m=j: emit_strip(2, m))
                hooks1.setdefault(21, []).append(lambda: emit_sums(2))
                av_phase(1, hooks1)

                av_phase(2, {})
    return nc


def _round_f32r(x):
    """Round-to-nearest-even to 11 mantissa bits (float32r semantics)."""
    u = np.ascontiguousarray(x, dtype=np.float32).view(np.uint32)
    rb = np.uint32(1 << 11)
    mask = np.uint32(0xFFFFF000)
    return ((u + rb) & mask).view(np.float32)


def kernel(left_features, right_features, wq, bq, wr, br):
    global LAST_RESULT
    left = np.asarray(left_features, dtype=np.float32)
    right = np.asarray(right_features, dtype=np.float32)
    wq = np.asarray(wq, dtype=np.float32)
    wr = np.asarray(wr, dtype=np.float32)
    bq = np.asarray(bq, dtype=np.float32)
    br = np.asarray(br, dtype=np.float32)

    lf = left.reshape(B, C, HW)
    rg = right.reshape(B, C, HW)
    lf_r = _round_f32r(lf)
    rg_r = _round_f32r(rg)
    # [C, D] -> [128, CB*D] with row p = concat over k of wq.T[k*128+p, :]
    wqp = np.ascontiguousarray(
        _round_f32r(wq.T).reshape(CB, 128, D).transpose(1, 0, 2)
        .reshape(128, CB * D))
    wrp = np.ascontiguousarray(
        _round_f32r(wr.T).reshape(CB, 128, D).transpose(1, 0, 2)
        .reshape(128, CB * D))
    bq_t = np.ascontiguousarray(bq.reshape(DB, 128).T)  # [128, DB]
    br_t = np.ascontiguousarray(br.reshape(DB, 128).T)

    nc = build_kernel()
    in_maps = []
    for core in range(NCORES):
        b, d = core // 2, core % 2
        qf_c = lf_r[b] if d == 0 else rg_r[b]
        rf_c = rg_r[b] if d == 0 else lf_r[b]
        in_maps.append({
            "qf": np.ascontiguousarray(qf_c),
            "rf": np.ascontiguousarray(rf_c),
            "vtb": np.ascontiguousarray(rf_c.T.astype(ml_dtypes.bfloat16)),
            "wqp": wqp, "wrp": wrp, "bq": bq_t, "br": br_t,
        })
    res = run_bass_kernel_spmd(nc, in_maps, core_ids=list(range(NCORES)),
                               trace=TRACE)
    LAST_RESULT = res

    weighted = np.stack(
        [np.ascontiguousarray(res.results[core]["outT"].T)
         for core in range(NCORES)])
    weighted = weighted.reshape(B, 2, C, 48, 48)
    left_att = np.concatenate([left, weighted[:, 0]], axis=1)
    right_att = np.concatenate([right, weighted[:, 1]], axis=1)
    return (left_att, right_att)


# revision 12
# speedup vs baseline: 1.0866x; 1.0866x over previous
"""CoAttention kernel for 8 TRN2 NeuronCores (Bass/Tile, SPMD).

Problem: B=4 batches x 2 attention directions = 8 independent co-attention
computations -> one per core.  Per core (batch b, direction d):
    Q = wq @ qf + bq        [256, 2304]     (qf = query-side features)
    K = wr @ rf + br        [256, 2304]     (rf = reference-side features)
    S^T = K^T Q             [2304, 2304]    (computed in m-strips of 128)
    attnT = exp(S^T - 40)   (bf16, unnormalized; softmax denom applied at end)
    sums[q] = sum_m attnT[m, q]
    outT = (attnT^T @ rf^T) * (1/sums)      [2304, 2048]  (host transposes)
Host assembles: left_att = concat(left, out[b,dir=0]), right_att likewise.

Precision: score path in float32r (RNE-11-mantissa, full PE rate at free>=256,
host pre-rounds inputs), attn@V in bf16.  No row-max subtraction: scores are
|S| <~ 80, exp(S-40) stays in fp32/bf16 range; normalization is exact math.

Schedule (v2): the PE instruction stream is kept dense end-to-end:
 - weights host-pre-permuted to [128, CB*D] so DMA rows are contiguous per
   partition; first weight/x blocks loaded as small DMAs so matmuls start ~9us
   instead of ~26us; loads split across sync (xq), scalar (xr, vt), gpsimd
   (w, stores) queues.
 - S^T strips for qt=0 are interleaved into the projection phase (they have no
   DMA dependency and fill x-stream stall gaps); strips for qt+1 are
   interleaved into av(qt) chains.
 - attn@V computed transposed: lhsT = attnT strip (weight), rhs = V^T strip
   (moving) -> outT [q, c] in 6 q-blocks x 4 c-chunks of 512, every matmul at
   full 512 moving columns.  Normalization is then a per-PARTITION scalar
   multiply: sums come from 6 single-column matmuls against the DVE-maintained
   strip-sum accumulator, and 1/sums is a [128,6] DVE reciprocal (~50ns)
   instead of a [1,768] single-lane one (~5us).  No broadcast tile, no DRAM
   round-trip.

Walrus in this toolchain allows ONE sync-wait per instruction; SafeTileContext
splits multi-wait instructions into standalone wait ops, and splits the
end-of-kernel drain the same way.
"""
import numpy as np
import ml_dtypes

import concourse.bass as bass
import concourse.mybir as mybir
import concourse.tile as tile
from concourse.vector_clock import ScopedClock
from concourse.bass_utils import run_bass_kernel_spmd

B = 4
C = 2048
HW = 48 * 48          # 2304
D = 256
NCORES = 8

CB = C // 128         # 16 c-blocks
DB = D // 128         # 2 d-blocks
MS = HW // 128        # 18 m-strips
# phase-1 n chunks: 512-wide (max f32r moving free dim) + 256 tail;
# each chunk's psum fits one 2KB PSUM bank
P1CHUNKS = [(0, 512), (512, 512), (1024, 512), (1536, 512), (2048, 256)]
NQT = 3               # phase-2 q thirds
QT = HW // NQT        # 768
QB = QT // 128        # 6 q-blocks per third
NCC = C // 512        # 4 c-chunks per av output row-block
# sub-chunks within a q-third for the score matmuls (a matmul output must not
# cross a 2KB PSUM bank boundary; both >=256 keeps f32r at full rate)
SUBS = [(0, 512), (512, 256)]

# S^T strips of qt=0 interleaved into projection: (chunk, pair) -> strips
# emitted before that x-pair's matmuls (strip m needs K columns from chunk
# m*128//512, and Q columns 0:768 from chunks 0-1)
STRIPS_AT = {
    (2, 0): (0,), (2, 1): (1,), (2, 2): (2,), (2, 3): (3,),
    (2, 4): (4,), (2, 5): (5,), (2, 6): (6,), (2, 7): (7,),
    (3, 0): (8,), (3, 2): (9,), (3, 4): (10,), (3, 6): (11,),
    (4, 0): (12,), (4, 2): (13,), (4, 4): (14,), (4, 6): (15,),
}
# vt stripe DMAs (one m-slice each, 512KB / 128 4KB-row descriptors: small
# enough not to displace the x stream in the DMA engines' descriptor FIFOs)
# emitted after these (chunk, pair)'s matmuls, on the gpsimd queue
VT_AT = {}
for _i in range(18):
    _ch, _pair = 1 + _i // 6, (_i % 6) + 1
    VT_AT[(_ch, _pair)] = _i

F32 = mybir.dt.float32
F32R = mybir.dt.float32r
BF16 = mybir.dt.bfloat16

# module-level knobs / results (used by test.py)
TRACE = False
LAST_RESULT = None


class SafeTileContext(tile.TileContext):
    """This walrus build allows at most ONE sync wait per instruction.
    Hoist extra waits onto standalone EventSemaphore (wait-only) ops placed
    immediately before, on the same engine queue; same for the final drain."""
    MAX_WAITS = 1

    def _lower_ordered_insts(self, ordered):
        for bname, insts in ordered.items():
            new_list = []
            for inst in insts:
                si = inst.sync_info
                if si is not None and len(si.on_wait) > self.MAX_WAITS:
                    waits = list(si.on_wait)
                    movable = [w for w in waits if w.wait_reg is None]
                    fixed = [w for w in waits if w.wait_reg is not None]
                    keep = fixed + movable[-1:] if movable else fixed
                    hoist = movable[:-1] if movable else []
                    for w in hoist:
                        wi = mybir.InstEventSemaphore(
                            name=self.nc.get_next_instruction_name(),
                            ins=[], outs=[])
                        wi.engine = inst.engine
                        wi.sync_info = mybir.SyncInfo(on_wait=[w], on_update=[])
                        new_list.append(wi)
                    inst.sync_info = mybir.SyncInfo(
                        on_wait=keep, on_update=list(si.on_update))
                new_list.append(inst)
            insts[:] = new_list
        super()._lower_ordered_insts(ordered)

    def _drain_and_barrier(self, tick_clock, wait_clock):
        drain_inst = self.nc.sync.drain()
        wait_clock.add_sem_waits(
            drain_inst.ins, ScopedClock({None: tick_clock.global_clock}))
        si = drain_inst.ins.sync_info
        waits = list(si.on_wait) if si is not None else []
        ups = list(si.on_update) if si is not None else []
        if len(waits) > self.MAX_WAITS:
            drain_inst.ins.sync_info = mybir.SyncInfo(
                on_wait=waits[: self.MAX_WAITS], on_update=ups)
            rest = waits[self.MAX_WAITS:]
            for i in range(0, len(rest), self.MAX_WAITS):
                extra = self.nc.sync.drain()
                extra.ins.sync_info = mybir.SyncInfo(
                    on_wait=rest[i : i + self.MAX_WAITS], on_update=[])
        self.nc.all_engine_barrier()
        assert self.sems is not None
        popped = self.nc._tile_sem_poison_stack.pop()
        assert popped is self._sem_poison
        self.nc.clear_and_free_semaphores(list(self.sems.allocated().values()))
        self.nc.all_engine_barrier()


def build_kernel():
    nc = bass.Bass("TRN2", target_bir_lowering=False, debug=False)

    qf = nc.dram_tensor("qf", [C, HW], F32R, kind="ExternalInput")
    rf = nc.dram_tensor("rf", [C, HW], F32R, kind="ExternalInput")
    vtb = nc.dram_tensor("vtb", [HW, C], BF16, kind="ExternalInput")
    # weights host-permuted to [128, CB*D]: row p holds wq.T[k*128+p, :] for
    # k = 0..CB-1 -> per-partition contiguous DMA rows
    wqp = nc.dram_tensor("wqp", [128, CB * D], F32R, kind="ExternalInput")
    wrp = nc.dram_tensor("wrp", [128, CB * D], F32R, kind="ExternalInput")
    bq = nc.dram_tensor("bq", [128, DB], F32, kind="ExternalInput")
    br = nc.dram_tensor("br", [128, DB], F32, kind="ExternalInput")
    outT = nc.dram_tensor("outT", [HW, C], F32, kind="ExternalOutput")

    with SafeTileContext(nc) as tc:
        with tc.tile_pool(name="persist", bufs=1) as persist, \
             tc.tile_pool(name="attnpA", bufs=1) as attnpA, \
             tc.tile_pool(name="spsp", bufs=2, space="PSUM") as spsp:
            # ---- persistent tiles ----
            q_sb = persist.tile([128, DB, HW], F32R)    # Q  [d, n]
            k_sb = persist.tile([128, DB, HW], F32R)    # K  [d, n]
            vt = persist.tile([128, MS, C], BF16)       # V^T [m, c]
            bq_t = persist.tile([128, DB], F32)
            br_t = persist.tile([128, DB], F32)
            nc.sync.dma_start(out=bq_t, in_=bq.ap())
            nc.sync.dma_start(out=br_t, in_=br.ap())
            nbias = persist.tile([128, 1], F32)
            nc.vector.memset(nbias, -40.0)
            ones_col = persist.tile([128, 1], BF16)     # sums reduce rhs
            nc.vector.memset(ones_col, 1.0)
            partial = persist.tile([128, QT], F32)      # DVE strip-accumulator
            partial_r = persist.tile([128, QT], BF16)   # PE copy (free=1
            # matmuls are bf16: f32r is illegal there; one 2^-9 rounding of
            # the softmax denominator, ~0.2% common-mode, is well in budget)

            attn_cur = {}
            invs_cur = {}
            pools = {}   # filled once the phase-2 pools open

            def emit_strip(qt, m):
                """Score matmuls + exp + DVE partial-sum for one m-strip."""
                if m == 0:
                    pool = attnpA if qt % 2 == 0 else pools["attnpB"]
                    attn_cur[qt] = pool.tile(
                        [128, MS, QT], BF16,
                        tag="attnA" if qt % 2 == 0 else "attnB",
                        name=f"attnT_{qt}")
                attn_t = attn_cur[qt]
                sps = spsp.tile([128, QT], F32, tag="sps", name=f"sps_{qt}_{m}")
                for off, sz in SUBS:
                    for dd in range(DB):
                        nc.tensor.matmul(
                            sps[:, off:off + sz],
                            k_sb[:, dd, m * 128:(m + 1) * 128],
                            q_sb[:, dd, qt * QT + off:qt * QT + off + sz],
                            start=(dd == 0), stop=(dd == DB - 1))
                nc.scalar.activation(
                    attn_t[:, m, :], sps,
                    mybir.ActivationFunctionType.Exp,
                    bias=nbias, scale=1.0)
                if m == 0:
                    nc.vector.tensor_copy(partial, attn_t[:, m, :])
                else:
                    nc.vector.tensor_add(partial, attn_t[:, m, :], partial)

            def emit_sums(qt):
                """partial [128,QT] -> per-q-block sums [128q, 1] (6 tiny
                matmuls) -> 1/sums [128, QB] via one cheap DVE reciprocal."""
                opsp = pools["opsp"]
                small = pools["small"]
                nc.vector.tensor_copy(partial_r, partial)
                aux = opsp.tile([128, 8], F32, tag="aux", name=f"aux_{qt}")
                for qb in range(QB):
                    nc.tensor.matmul(
                        aux[:, qb:qb + 1],
                        partial_r[:, qb * 128:(qb + 1) * 128],
                        ones_col, start=True, stop=True)
                invs_t = small.tile([128, 8], F32, tag="invs",
                                    name=f"invs_{qt}")
                nc.vector.reciprocal(invs_t[:, 0:QB], aux[:, 0:QB])
                invs_cur[qt] = invs_t

            # ================= phase 1: projections + S^T(qt0) ============
            with tc.tile_pool(name="wpool", bufs=1) as wpool, \
                 tc.tile_pool(name="xstream", bufs=4) as xstream, \
                 tc.tile_pool(name="p1ps", bufs=1, space="PSUM") as p1ps:
                wq_sb = wpool.tile([128, CB, D], F32R)
                wr_sb = wpool.tile([128, CB, D], F32R)
                # weights on gpsimd queue, first blocks small so matmuls can
                # start immediately
                for c0, c1 in ((0, 2), (2, 4), (4, 10), (10, 16)):
                    nc.gpsimd.dma_start(out=wq_sb[:, c0:c1, :],
                                        in_=wqp.ap()[:, c0 * D:c1 * D])
                    nc.gpsimd.dma_start(out=wr_sb[:, c0:c1, :],
                                        in_=wrp.ap()[:, c0 * D:c1 * D])

                qfr = qf.ap().rearrange("(k p) n -> p k n", p=128)
                rfr = rf.ap().rearrange("(k p) n -> p k n", p=128)
                vtr = vtb.ap().rearrange("(s p) c -> p s c", p=128)

                for ch, (coff, csz) in enumerate(P1CHUNKS):
                    qps = [p1ps.tile([128, 512], F32, tag=f"qps{d}",
                                     name=f"qps{d}_{ch}")
                           for d in range(DB)]
                    kps = [p1ps.tile([128, 512], F32, tag=f"kps{d}",
                                     name=f"kps{d}_{ch}")
                           for d in range(DB)]
                    for pair in range(CB // 2):
                        xq = xstream.tile([128, 2, 512], F32R, tag="xq",
                                          name=f"xq_{ch}_{pair}")
                        xr = xstream.tile([128, 2, 512], F32R, tag="xr",
                                          name=f"xr_{ch}_{pair}")
                        nc.sync.dma_start(
                            out=xq[:, :, :csz],
                            in_=qfr[:, pair * 2:(pair + 1) * 2,
                                    coff:coff + csz])
                        nc.scalar.dma_start(
                            out=xr[:, :, :csz],
                            in_=rfr[:, pair * 2:(pair + 1) * 2,
                                    coff:coff + csz])
                        for m in STRIPS_AT.get((ch, pair), ()):
                            emit_strip(0, m)
                        for i in range(2):
                            c = pair * 2 + i
                            for d in range(DB):
                                nc.tensor.matmul(
                                    qps[d][:, :csz],
                                    wq_sb[:, c, d * 128:(d + 1) * 128],
                                    xq[:, i, :csz],
                                    start=(c == 0), stop=(c == CB - 1))
                                nc.tensor.matmul(
                                    kps[d][:, :csz],
                                    wr_sb[:, c, d * 128:(d + 1) * 128],
                                    xr[:, i, :csz],
                                    start=(c == 0), stop=(c == CB - 1))
                        h = VT_AT.get((ch, pair))
                        if h is not None:
                            nc.gpsimd.dma_start(
                                out=vt[:, h:h + 1, :],
                                in_=vtr[:, h:h + 1, :])
                    # k-side bias first: the last S^T strips wait on it
                    for d in range(DB):
                        nc.vector.tensor_scalar_add(
                            k_sb[:, d, coff:coff + csz],
                            kps[d][:, :csz], br_t[:, d:d + 1])
                    for d in range(DB):
                        nc.vector.tensor_scalar_add(
                            q_sb[:, d, coff:coff + csz],
                            qps[d][:, :csz], bq_t[:, d:d + 1])
                # last strips need the final chunk's bias-adds
                emit_strip(0, 16)
                emit_strip(0, 17)

            # ================= phase 2: av(qt) + S^T(qt+1) ================
            with tc.tile_pool(name="attnpB", bufs=1) as attnpB, \
                 tc.tile_pool(name="small", bufs=2) as small, \
                 tc.tile_pool(name="ostage", bufs=3) as ostage, \
                 tc.tile_pool(name="opsp", bufs=2, space="PSUM") as opsp:
                pools["attnpB"] = attnpB
                pools["small"] = small
                pools["opsp"] = opsp

                # sums(0) (brief PE wait on the last strip's exp+add), then
                # 7 strips of qt1 emitted before av(0)
                emit_sums(0)
                for j in range(7):
                    emit_strip(1, j)

                def av_phase(qt, hooks):
                    attn_t = attn_cur.pop(qt)
                    invs_t = invs_cur.pop(qt)
                    last = (qt == NQT - 1)
                    ci = 0
                    for qb in range(QB):
                        oT = ostage.tile([128, C], F32, tag="osb",
                                         name=f"osb_{qt}_{qb}")
                        for cc in range(NCC):
                            for fn in hooks.get(ci, ()):
                                fn()
                            ci += 1
                            ops = opsp.tile([128, 512], F32, tag="ops")
                            for m in range(MS):
                                nc.tensor.matmul(
                                    ops,
                                    attn_t[:, m, qb * 128:(qb + 1) * 128],
                                    vt[:, m, cc * 512:(cc + 1) * 512],
                                    start=(m == 0), stop=(m == MS - 1))
                            nc.vector.tensor_scalar_mul(
                                oT[:, cc * 512:(cc + 1) * 512], ops,
                                invs_t[:, qb:qb + 1])
                            if last and qb >= 4:
                                nc.gpsimd.dma_start(
                                    out=outT.ap()[qt * QT + qb * 128:
                                                  qt * QT + (qb + 1) * 128,
                                                  cc * 512:(cc + 1) * 512],
                                    in_=oT[:, cc * 512:(cc + 1) * 512])
                        if not (last and qb >= 4):
                            nc.gpsimd.dma_start(
                                out=outT.ap()[qt * QT + qb * 128:
                                              qt * QT + (qb + 1) * 128, :],
                                in_=oT)

                # av(0): remaining strips of qt1 at chains 1..11, sums(1) at
                # 14 (after qt1 exps/adds land, invs ready long before av(1))
                hooks0 = {}
                for j in range(7, MS):
                    hooks0.setdefault(1 + (j - 7), []).append(
                        lambda m=j: emit_strip(1, m))
                hooks0.setdefault(14, []).append(lambda: emit_sums(1))
                av_phase(0, hooks0)

                hooks1 = {}
                for j in range(MS):
                    hooks1.setdefault(1 + j, []).append(
                        lambda m=j: emit_strip(2, m))
                hooks1.setdefault(21, []).append(lambda: emit_sums(2))
                av_phase(1, hooks1)

                av_phase(2, {})
    return nc


def _round_f32r(x):
    """Round-to-nearest-even to 11 mantissa bits (float32r semantics)."""
    u = np.ascontiguousarray(x, dtype=np.float32).view(np.uint32)
    rb = np.uint32(1 << 11)
    mask = np.uint32(0xFFFFF000)
    return ((u + rb) & mask).view(np.float32)


def kernel(left_features, right_features, wq, bq, wr, br):
    global LAST_RESULT
    left = np.asarray(left_features, dtype=np.float32)
    right = np.asarray(right_features, dtype=np.float32)
    wq = np.asarray(wq, dtype=np.float32)
    wr = np.asarray(wr, dtype=np.float32)
    bq = np.asarray(bq, dtype=np.float32)
    br = np.asarray(br, dtype=np.float32)

    lf = left.reshape(B, C, HW)
    rg = right.reshape(B, C, HW)
    lf_r = _round_f32r(lf)
    rg_r = _round_f32r(rg)
    # [C, D] -> [128, CB*D] with row p = concat over k of wq.T[k*128+p, :]
    wqp = np.ascontiguousarray(
        _round_f32r(wq.T).reshape(CB, 128, D).transpose(1, 0, 2)
        .reshape(128, CB * D))
    wrp = np.ascontiguousarray(
        _round_f32r(wr.T).reshape(CB, 128, D).transpose(1, 0, 2)
        .reshape(128, CB * D))
    bq_t = np.ascontiguousarray(bq.reshape(DB, 128).T)  # [128, DB]
    br_t = np.ascontiguousarray(br.reshape(DB, 128).T)

    nc = build_kernel()
    in_maps = []
    for core in range(NCORES):
        b, d = core // 2, core % 2
        qf_c = lf_r[b] if d == 0 else rg_r[b]
        rf_c = rg_r[b] if d == 0 else lf_r[b]
        in_maps.append({
            "qf": np.ascontiguousarray(qf_c),
            "rf": np.ascontiguousarray(rf_c),
            "vtb": np.ascontiguousarray(rf_c.T.astype(ml_dtypes.bfloat16)),
            "wqp": wqp, "wrp": wrp, "bq": bq_t, "br": br_t,
        })
    res = run_bass_kernel_spmd(nc, in_maps, core_ids=list(range(NCORES)),
                               trace=TRACE)
    LAST_RESULT = res

    weighted = np.stack(
        [np.ascontiguousarray(res.results[core]["outT"].T)
         for core in range(NCORES)])
    weighted = weighted.reshape(B, 2, C, 48, 48)
    left_att = np.concatenate([left, weighted[:, 0]], axis=1)
    right_att = np.concatenate([right, weighted[:, 1]], axis=1)
    return (left_att, right_att)


# revision 15
# speedup vs baseline: 1.0879x; 1.0012x over previous
"""CoAttention kernel for 8 TRN2 NeuronCores (Bass/Tile, SPMD).

Problem: B=4 batches x 2 attention directions = 8 independent co-attention
computations -> one per core.  Per core (batch b, direction d):
    Q = wq @ qf + bq        [256, 2304]     (qf = query-side features)
    K = wr @ rf + br        [256, 2304]     (rf = reference-side features)
    S^T = K^T Q             [2304, 2304]    (computed in m-strips of 128)
    attnT = exp(S^T - 40)   (bf16, unnormalized; softmax denom applied at end)
    sums[q] = sum_m attnT[m, q]
    outT = (attnT^T @ rf^T) * (1/sums)      [2304, 2048]  (host transposes)
Host assembles: left_att = concat(left, out[b,dir=0]), right_att likewise.

Precision: score path in float32r (RNE-11-mantissa, full PE rate at free>=256,
host pre-rounds inputs), attn@V in bf16.  No row-max subtraction: scores are
|S| <~ 80, exp(S-40) stays in fp32/bf16 range; normalization is exact math.

Schedule (v2): the PE instruction stream is kept dense end-to-end:
 - weights host-pre-permuted to [128, CB*D] so DMA rows are contiguous per
   partition; first weight/x blocks loaded as small DMAs so matmuls start ~9us
   instead of ~26us; loads split across sync (xq), scalar (xr, vt), gpsimd
   (w, stores) queues.
 - S^T strips for qt=0 are interleaved into the projection phase (they have no
   DMA dependency and fill x-stream stall gaps); strips for qt+1 are
   interleaved into av(qt) chains.
 - attn@V computed transposed: lhsT = attnT strip (weight), rhs = V^T strip
   (moving) -> outT [q, c] in 6 q-blocks x 4 c-chunks of 512, every matmul at
   full 512 moving columns.  Normalization is then a per-PARTITION scalar
   multiply: sums come from 6 single-column matmuls against the DVE-maintained
   strip-sum accumulator, and 1/sums is a [128,6] DVE reciprocal (~50ns)
   instead of a [1,768] single-lane one (~5us).  No broadcast tile, no DRAM
   round-trip.

Walrus in this toolchain allows ONE sync-wait per instruction; SafeTileContext
splits multi-wait instructions into standalone wait ops, and splits the
end-of-kernel drain the same way.
"""
import numpy as np
import ml_dtypes

import concourse.bass as bass
import concourse.mybir as mybir
import concourse.tile as tile
from concourse.vector_clock import ScopedClock
from concourse.bass_utils import run_bass_kernel_spmd

B = 4
C = 2048
HW = 48 * 48          # 2304
D = 256
NCORES = 8

CB = C // 128         # 16 c-blocks
DB = D // 128         # 2 d-blocks
MS = HW // 128        # 18 m-strips
# phase-1 n chunks: 512-wide (max f32r moving free dim) + 256 tail;
# each chunk's psum fits one 2KB PSUM bank
P1CHUNKS = [(0, 512), (512, 512), (1024, 512), (1536, 512), (2048, 256)]
NQT = 3               # phase-2 q thirds
QT = HW // NQT        # 768
QB = QT // 128        # 6 q-blocks per third
NCC = C // 512        # 4 c-chunks per av output row-block
# sub-chunks within a q-third for the score matmuls (a matmul output must not
# cross a 2KB PSUM bank boundary; both >=256 keeps f32r at full rate)
SUBS = [(0, 512), (512, 256)]

# S^T strips of qt=0 interleaved into projection: (chunk, pair) -> strips
# emitted before that x-pair's matmuls (strip m needs K columns from chunk
# m*128//512, and Q columns 0:768 from chunks 0-1)
STRIPS_AT = {
    (2, 0): (0,), (2, 1): (1,), (2, 2): (2,), (2, 3): (3,),
    (2, 4): (4,), (2, 5): (5,), (2, 6): (6,), (2, 7): (7,),
    (3, 0): (8,), (3, 2): (9,), (3, 4): (10,), (3, 6): (11,),
    (4, 0): (12,), (4, 2): (13,), (4, 4): (14,), (4, 6): (15,),
}
# Q-side projection windows deferred into av(0): phase 1 only projects Q for
# columns 0:1024 (scores(qt0) need 0:768); these windows run inside av(0)
QDEFER = [(1024, 512), (1536, 512), (2048, 256)]

F32 = mybir.dt.float32
F32R = mybir.dt.float32r
BF16 = mybir.dt.bfloat16

# module-level knobs / results (used by test.py)
TRACE = False
LAST_RESULT = None


class SafeTileContext(tile.TileContext):
    """This walrus build allows at most ONE sync wait per instruction.
    Hoist extra waits onto standalone EventSemaphore (wait-only) ops placed
    immediately before, on the same engine queue; same for the final drain."""
    MAX_WAITS = 1

    def _lower_ordered_insts(self, ordered):
        for bname, insts in ordered.items():
            new_list = []
            for inst in insts:
                si = inst.sync_info
                if si is not None and len(si.on_wait) > self.MAX_WAITS:
                    waits = list(si.on_wait)
                    movable = [w for w in waits if w.wait_reg is None]
                    fixed = [w for w in waits if w.wait_reg is not None]
                    keep = fixed + movable[-1:] if movable else fixed
                    hoist = movable[:-1] if movable else []
                    for w in hoist:
                        wi = mybir.InstEventSemaphore(
                            name=self.nc.get_next_instruction_name(),
                            ins=[], outs=[])
                        wi.engine = inst.engine
                        wi.sync_info = mybir.SyncInfo(on_wait=[w], on_update=[])
                        new_list.append(wi)
                    inst.sync_info = mybir.SyncInfo(
                        on_wait=keep, on_update=list(si.on_update))
                new_list.append(inst)
            insts[:] = new_list
        super()._lower_ordered_insts(ordered)

    def _drain_and_barrier(self, tick_clock, wait_clock):
        drain_inst = self.nc.sync.drain()
        wait_clock.add_sem_waits(
            drain_inst.ins, ScopedClock({None: tick_clock.global_clock}))
        si = drain_inst.ins.sync_info
        waits = list(si.on_wait) if si is not None else []
        ups = list(si.on_update) if si is not None else []
        if len(waits) > self.MAX_WAITS:
            drain_inst.ins.sync_info = mybir.SyncInfo(
                on_wait=waits[: self.MAX_WAITS], on_update=ups)
            rest = waits[self.MAX_WAITS:]
            for i in range(0, len(rest), self.MAX_WAITS):
                extra = self.nc.sync.drain()
                extra.ins.sync_info = mybir.SyncInfo(
                    on_wait=rest[i : i + self.MAX_WAITS], on_update=[])
        self.nc.all_engine_barrier()
        assert self.sems is not None
        popped = self.nc._tile_sem_poison_stack.pop()
        assert popped is self._sem_poison
        self.nc.clear_and_free_semaphores(list(self.sems.allocated().values()))
        self.nc.all_engine_barrier()


def build_kernel():
    nc = bass.Bass("TRN2", target_bir_lowering=False, debug=False)

    qf = nc.dram_tensor("qf", [C, HW], F32R, kind="ExternalInput")
    rf = nc.dram_tensor("rf", [C, HW], F32R, kind="ExternalInput")
    vtb = nc.dram_tensor("vtb", [HW, C], BF16, kind="ExternalInput")
    # weights host-permuted to [128, CB*D]: row p holds wq.T[k*128+p, :] for
    # k = 0..CB-1 -> per-partition contiguous DMA rows
    wqp = nc.dram_tensor("wqp", [128, CB * D], F32R, kind="ExternalInput")
    wrp = nc.dram_tensor("wrp", [128, CB * D], F32R, kind="ExternalInput")
    bq = nc.dram_tensor("bq", [128, DB], F32, kind="ExternalInput")
    br = nc.dram_tensor("br", [128, DB], F32, kind="ExternalInput")
    outT = nc.dram_tensor("outT", [HW, C], F32, kind="ExternalOutput")

    with SafeTileContext(nc) as tc:
        with tc.tile_pool(name="persist", bufs=1) as persist, \
             tc.tile_pool(name="attnpA", bufs=1) as attnpA, \
             tc.tile_pool(name="spsp", bufs=2, space="PSUM") as spsp:
            # ---- persistent tiles ----
            q_sb = persist.tile([128, DB, HW], F32R)    # Q  [d, n]
            k_sb = persist.tile([128, DB, HW], F32R)    # K  [d, n]
            vt = persist.tile([128, MS, C], BF16)       # V^T [m, c]
            wq_sb = persist.tile([128, CB, D], F32R)    # wq (used into av0)
            bq_t = persist.tile([128, DB], F32)
            br_t = persist.tile([128, DB], F32)
            nc.sync.dma_start(out=bq_t, in_=bq.ap())
            nc.sync.dma_start(out=br_t, in_=br.ap())
            nbias = persist.tile([128, 1], F32)
            nc.vector.memset(nbias, -40.0)
            ones_col = persist.tile([128, 1], BF16)     # sums reduce rhs
            nc.vector.memset(ones_col, 1.0)
            partial = persist.tile([128, QT], F32)      # DVE strip-accumulator
            partial_r = persist.tile([128, QT], BF16)   # PE copy (free=1
            # matmuls are bf16: f32r is illegal there; one 2^-9 rounding of
            # the softmax denominator, ~0.2% common-mode, is well in budget)

            qfr = qf.ap().rearrange("(k p) n -> p k n", p=128)
            vtr = vtb.ap().rearrange("(s p) c -> p s c", p=128)

            attn_cur = {}
            invs_cur = {}
            pools = {}   # filled once the phase-2 pools open

            def emit_strip(qt, m):
                """Score matmuls + exp + DVE partial-sum for one m-strip."""
                if m == 0:
                    pool = attnpA if qt % 2 == 0 else pools["attnpB"]
                    attn_cur[qt] = pool.tile(
                        [128, MS, QT], BF16,
                        tag="attnA" if qt % 2 == 0 else "attnB",
                        name=f"attnT_{qt}")
                attn_t = attn_cur[qt]
                sps = spsp.tile([128, QT], F32, tag="sps", name=f"sps_{qt}_{m}")
                for off, sz in SUBS:
                    for dd in range(DB):
                        nc.tensor.matmul(
                            sps[:, off:off + sz],
                            k_sb[:, dd, m * 128:(m + 1) * 128],
                            q_sb[:, dd, qt * QT + off:qt * QT + off + sz],
                            start=(dd == 0), stop=(dd == DB - 1))
                nc.scalar.activation(
                    attn_t[:, m, :], sps,
                    mybir.ActivationFunctionType.Exp,
                    bias=nbias, scale=1.0)
                if m == 0:
                    nc.vector.tensor_copy(partial, attn_t[:, m, :])
                else:
                    nc.vector.tensor_add(partial, attn_t[:, m, :], partial)

            def emit_sums(qt):
                """partial [128,QT] -> per-q-block sums [128q, 1] (6 tiny
                matmuls) -> 1/sums [128, QB] via one cheap DVE reciprocal."""
                opsp = pools["opsp"]
                small = pools["small"]
                nc.vector.tensor_copy(partial_r, partial)
                aux = opsp.tile([128, 512], F32, tag="aux", name=f"aux_{qt}")
                for qb in range(QB):
                    nc.tensor.matmul(
                        aux[:, qb:qb + 1],
                        partial_r[:, qb * 128:(qb + 1) * 128],
                        ones_col, start=True, stop=True)
                invs_t = small.tile([128, 8], F32, tag="invs",
                                    name=f"invs_{qt}")
                nc.vector.reciprocal(invs_t[:, 0:QB], aux[:, 0:QB])
                invs_cur[qt] = invs_t

            def emit_qwin_dma(woff, wsz, pairs):
                x2p = pools["x2"]
                xt = pools.setdefault("x2tiles", {})
                for pair in pairs:
                    x2 = x2p.tile([128, 2, 512], F32R, tag="x2",
                                  name=f"x2_{woff}_{pair}")
                    xt[(woff, pair)] = x2
                    nc.sync.dma_start(
                        out=x2[:, :, :wsz],
                        in_=qfr[:, pair * 2:(pair + 1) * 2, woff:woff + wsz])

            def emit_qwin_mms(woff, wsz, pairs, auxs):
                """Deferred Q-projection matmuls for the given x-pairs of
                window (woff, wsz) into the aux psum pair."""
                opsp = pools["opsp"]
                xt = pools["x2tiles"]
                for pair in pairs:
                    if pair == 0:
                        for d in range(DB):
                            auxs[d] = opsp.tile([128, 512], F32, tag="aux",
                                                name=f"qw{woff}_{d}")
                    x2 = xt.pop((woff, pair))
                    for i in range(2):
                        c = pair * 2 + i
                        for d in range(DB):
                            nc.tensor.matmul(
                                auxs[d][:, :wsz],
                                wq_sb[:, c, d * 128:(d + 1) * 128],
                                x2[:, i, :wsz],
                                start=(c == 0), stop=(c == CB - 1))

            def emit_qwin_bias(woff, wsz, auxs):
                for d in range(DB):
                    nc.vector.tensor_scalar_add(
                        q_sb[:, d, woff:woff + wsz],
                        auxs[d][:, :wsz], bq_t[:, d:d + 1])

            # ====== phase 1: K-proj (all cols) + Q-proj (cols 0:1024) ======
            # + S^T(qt0) strips + vt c-chunk 0
            with tc.tile_pool(name="wpool", bufs=1) as wpool, \
                 tc.tile_pool(name="xstream", bufs=4) as xstream, \
                 tc.tile_pool(name="p1ps", bufs=1, space="PSUM") as p1ps:
                wr_sb = wpool.tile([128, CB, D], F32R)
                # weights on gpsimd queue, first blocks small so matmuls can
                # start immediately
                for c0, c1 in ((0, 2), (2, 4), (4, 10), (10, 16)):
                    nc.gpsimd.dma_start(out=wq_sb[:, c0:c1, :],
                                        in_=wqp.ap()[:, c0 * D:c1 * D])
                    nc.gpsimd.dma_start(out=wr_sb[:, c0:c1, :],
                                        in_=wrp.ap()[:, c0 * D:c1 * D])

                rfr = rf.ap().rearrange("(k p) n -> p k n", p=128)

                for ch, (coff, csz) in enumerate(P1CHUNKS):
                    do_q = ch < 2
                    if do_q:
                        qps = [p1ps.tile([128, 512], F32, tag=f"qps{d}",
                                         name=f"qps{d}_{ch}")
                               for d in range(DB)]
                    kps = [p1ps.tile([128, 512], F32, tag=f"kps{d}",
                                     name=f"kps{d}_{ch}")
                           for d in range(DB)]
                    for pair in range(CB // 2):
                        if do_q:
                            xq = xstream.tile([128, 2, 512], F32R, tag="xq",
                                              name=f"xq_{ch}_{pair}")
                            nc.sync.dma_start(
                                out=xq[:, :, :csz],
                                in_=qfr[:, pair * 2:(pair + 1) * 2,
                                        coff:coff + csz])
                        xr = xstream.tile([128, 2, 512], F32R, tag="xr",
                                          name=f"xr_{ch}_{pair}")
                        nc.scalar.dma_start(
                            out=xr[:, :, :csz],
                            in_=rfr[:, pair * 2:(pair + 1) * 2,
                                    coff:coff + csz])
                        for m in STRIPS_AT.get((ch, pair), ()):
                            emit_strip(0, m)
                        for i in range(2):
                            c = pair * 2 + i
                            for d in range(DB):
                                if do_q:
                                    nc.tensor.matmul(
                                        qps[d][:, :csz],
                                        wq_sb[:, c, d * 128:(d + 1) * 128],
                                        xq[:, i, :csz],
                                        start=(c == 0), stop=(c == CB - 1))
                                nc.tensor.matmul(
                                    kps[d][:, :csz],
                                    wr_sb[:, c, d * 128:(d + 1) * 128],
                                    xr[:, i, :csz],
                                    start=(c == 0), stop=(c == CB - 1))
                        # vt c-chunk 0 after the x stream thins out
                        if (ch, pair) == (3, 4):
                            nc.gpsimd.dma_start(out=vt[:, :, 0:512],
                                                in_=vtr[:, :, 0:512])
                    # k-side bias first: the last S^T strips wait on it
                    for d in range(DB):
                        nc.vector.tensor_scalar_add(
                            k_sb[:, d, coff:coff + csz],
                            kps[d][:, :csz], br_t[:, d:d + 1])
                    if do_q:
                        for d in range(DB):
                            nc.vector.tensor_scalar_add(
                                q_sb[:, d, coff:coff + csz],
                                qps[d][:, :csz], bq_t[:, d:d + 1])
                # last strips need the final chunk's bias-adds
                emit_strip(0, 16)
                emit_strip(0, 17)

            # ================= phase 2: av(qt) + S^T(qt+1) ================
            with tc.tile_pool(name="attnpB", bufs=1) as attnpB, \
                 tc.tile_pool(name="small", bufs=2) as small, \
                 tc.tile_pool(name="ostage", bufs=3) as ostage, \
                 tc.tile_pool(name="x2", bufs=3) as x2p, \
                 tc.tile_pool(name="opsp", bufs=2, space="PSUM") as opsp:
                pools["attnpB"] = attnpB
                pools["small"] = small
                pools["opsp"] = opsp
                pools["x2"] = x2p

                emit_sums(0)

                def av_phase(qt, hooks):
                    attn_t = attn_cur.pop(qt)
                    invs_t = invs_cur.pop(qt)
                    ci = 0
                    for cc in range(NCC):
                        for qb in range(QB):
                            for fn in hooks.get(ci, ()):
                                fn()
                            ci += 1
                            ops = opsp.tile([128, 512], F32, tag="ops")
                            for m in range(MS):
                                nc.tensor.matmul(
                                    ops,
                                    attn_t[:, m, qb * 128:(qb + 1) * 128],
                                    vt[:, m, cc * 512:(cc + 1) * 512],
                                    start=(m == 0), stop=(m == MS - 1))
                            o_sb = ostage.tile([128, 512], F32, tag="osb",
                                               name=f"osb_{qt}_{cc}_{qb}")
                            nc.vector.tensor_scalar_mul(
                                o_sb, ops, invs_t[:, qb:qb + 1])
                            nc.gpsimd.dma_start(
                                out=outT.ap()[qt * QT + qb * 128:
                                              qt * QT + (qb + 1) * 128,
                                              cc * 512:(cc + 1) * 512],
                                in_=o_sb)

                # av(0) hooks: deferred Q-projection windows (A: 1024:
                # 1536, B: 1536:2048, C: 2048:2304) with x-pair DMAs staged
                # one hook ahead of their matmuls; vt c-chunks 1-3; strips of
                # qt1 (2 per hook, after window-A bias lands); sums(1)
                auxA, auxB, auxC = {}, {}, {}
                (wAo, wAs), (wBo, wBs), (wCo, wCs) = QDEFER
                hooks0 = {}

                def at(ci, fn):
                    hooks0.setdefault(ci, []).append(fn)

                at(0, lambda: nc.gpsimd.dma_start(out=vt[:, :, 512:1024],
                                                  in_=vtr[:, :, 512:1024]))
                at(0, lambda: emit_qwin_dma(wAo, wAs, (0, 1, 2)))
                at(1, lambda: emit_qwin_dma(wAo, wAs, (3, 4)))
                at(1, lambda: emit_qwin_mms(wAo, wAs, (0, 1), auxA))
                at(2, lambda: emit_qwin_dma(wAo, wAs, (5, 6)))
                at(2, lambda: emit_qwin_mms(wAo, wAs, (2, 3), auxA))
                at(3, lambda: emit_qwin_dma(wAo, wAs, (7,)))
                at(3, lambda: emit_qwin_mms(wAo, wAs, (4, 5), auxA))
                at(4, lambda: emit_qwin_mms(wAo, wAs, (6, 7), auxA))
                at(4, lambda: emit_qwin_dma(wBo, wBs, (0, 1)))
                at(5, lambda: emit_qwin_bias(wAo, wAs, auxA))
                at(5, lambda: emit_qwin_dma(wBo, wBs, (2, 3)))
                at(5, lambda: emit_qwin_mms(wBo, wBs, (0, 1), auxB))
                at(6, lambda: emit_qwin_dma(wBo, wBs, (4, 5)))
                at(6, lambda: emit_qwin_mms(wBo, wBs, (2, 3), auxB))
                at(7, lambda: emit_qwin_dma(wBo, wBs, (6, 7)))
                at(7, lambda: emit_qwin_mms(wBo, wBs, (4, 5), auxB))
                at(8, lambda: emit_qwin_mms(wBo, wBs, (6, 7), auxB))
                at(8, lambda: emit_qwin_dma(wCo, wCs, (0, 1)))
                at(9, lambda: emit_qwin_bias(wBo, wBs, auxB))
                at(9, lambda: emit_qwin_dma(wCo, wCs, (2, 3)))
                at(9, lambda: emit_qwin_mms(wCo, wCs, (0, 1), auxC))
                at(10, lambda: emit_qwin_dma(wCo, wCs, (4, 5)))
                at(10, lambda: emit_qwin_mms(wCo, wCs, (2, 3), auxC))
                at(11, lambda: emit_qwin_dma(wCo, wCs, (6, 7)))
                at(11, lambda: emit_qwin_mms(wCo, wCs, (4, 5), auxC))
                at(12, lambda: emit_qwin_mms(wCo, wCs, (6, 7), auxC))
                at(12, lambda: nc.gpsimd.dma_start(out=vt[:, :, 1024:1536],
                                                   in_=vtr[:, :, 1024:1536]))
                at(13, lambda: emit_qwin_bias(wCo, wCs, auxC))
                at(16, lambda: nc.gpsimd.dma_start(out=vt[:, :, 1536:2048],
                                                   in_=vtr[:, :, 1536:2048]))
                # strips(1) only need Q cols 768:1536 (phase 1 + window A)
                for j in range(MS):
                    at(5 + j // 2, lambda m=j: emit_strip(1, m))
                at(16, lambda: emit_sums(1))
                av_phase(0, hooks0)

                hooks1 = {}
                for j in range(MS):
                    hooks1.setdefault(1 + j, []).append(
                        lambda m=j: emit_strip(2, m))
                hooks1.setdefault(21, []).append(lambda: emit_sums(2))
                av_phase(1, hooks1)

                av_phase(2, {})
    return nc


def _round_f32r(x):
    """Round-to-nearest-even to 11 mantissa bits (float32r semantics)."""
    u = np.ascontiguousarray(x, dtype=np.float32).view(np.uint32)
    rb = np.uint32(1 << 11)
    mask = np.uint32(0xFFFFF000)
    return ((u + rb) & mask).view(np.float32)


def kernel(left_features, right_features, wq, bq, wr, br):
    global LAST_RESULT
    left = np.asarray(left_features, dtype=np.float32)
    right = np.asarray(right_features, dtype=np.float32)
    wq = np.asarray(wq, dtype=np.float32)
    wr = np.asarray(wr, dtype=np.float32)
    bq = np.asarray(bq, dtype=np.float32)
    br = np.asarray(br, dtype=np.float32)

    lf = left.reshape(B, C, HW)
    rg = right.reshape(B, C, HW)
    lf_r = _round_f32r(lf)
    rg_r = _round_f32r(rg)
    # [C, D] -> [128, CB*D] with row p = concat over k of wq.T[k*128+p, :]
    wqp = np.ascontiguousarray(
        _round_f32r(wq.T).reshape(CB, 128, D).transpose(1, 0, 2)
        .reshape(128, CB * D))
    wrp = np.ascontiguousarray(
        _round_f32r(wr.T).reshape(CB, 128, D).transpose(1, 0, 2)
        .reshape(128, CB * D))
    bq_t = np.ascontiguousarray(bq.reshape(DB, 128).T)  # [128, DB]
    br_t = np.ascontiguousarray(br.reshape(DB, 128).T)

    nc = build_kernel()
    in_maps = []
    for core in range(NCORES):
        b, d = core // 2, core % 2
        qf_c = lf_r[b] if d == 0 else rg_r[b]
        rf_c = rg_r[b] if d == 0 else lf_r[b]
        in_maps.append({
            "qf": np.ascontiguousarray(qf_c),
            "rf": np.ascontiguousarray(rf_c),
            "vtb": np.ascontiguousarray(rf_c.T.astype(ml_dtypes.bfloat16)),
            "wqp": wqp, "wrp": wrp, "bq": bq_t, "br": br_t,
        })
    res = run_bass_kernel_spmd(nc, in_maps, core_ids=list(range(NCORES)),
                               trace=TRACE)
    LAST_RESULT = res

    weighted = np.stack(
        [np.ascontiguousarray(res.results[core]["outT"].T)
         for core in range(NCORES)])
    weighted = weighted.reshape(B, 2, C, 48, 48)
    left_att = np.concatenate([left, weighted[:, 0]], axis=1)
    right_att = np.concatenate([right, weighted[:, 1]], axis=1)
    return (left_att, right_att)
